# revision 77
# baseline (speedup 1.0000x reference)
import sys
sys.path.insert(0, "/opt/trn_rl_repo")
import numpy as np
import ml_dtypes
import concourse.bacc as bacc
import concourse.bass as bass
import concourse.mybir as mybir
import concourse.tile as tile
from concourse.bass import ds, ts
from concourse.bass_utils import run_bass_kernel_spmd

BF = ml_dtypes.bfloat16
P = 128
NT = 577          # tokens
D = 768
H = 16
HD = 48
KO = 6            # 768 = 6*128 contraction tiles (bias handled separately)
NBLK = [(0, 128), (128, 128), (256, 128), (384, 128), (512, 65)]
MPAD = 580        # m padded to 4*145
NG = 145          # m4 groups
SCALE = HD ** -0.5
EXP = mybir.ActivationFunctionType.Exp

_cache = {}
ABLATE = set()


def _build(nc):
    bf = mybir.dt.bfloat16
    f32 = mybir.dt.float32
    x_d = nc.dram_tensor("xT", [D, NT], bf, kind="ExternalInput")
    xf_d = nc.dram_tensor("xfT", [D, NT], bf, kind="ExternalInput")
    wq_d = nc.dram_tensor("wqT", [D, 1024], bf, kind="ExternalInput")
    wk_d = nc.dram_tensor("wkT", [D, 1024], bf, kind="ExternalInput")
    wv_d = nc.dram_tensor("wvT", [D, 784], bf, kind="ExternalInput")
    qb_d = nc.dram_tensor("qb", [P, 8], f32, kind="ExternalInput")
    pw_d = nc.dram_tensor("pwT", [1024, D], bf, kind="ExternalInput")
    pwb_d = nc.dram_tensor("pwb", [P, 6], f32, kind="ExternalInput")
    wb_d = nc.dram_tensor("wblk", [P, 64], bf, kind="ExternalInput")
    id_d = nc.dram_tensor("idn", [P, P], bf, kind="ExternalInput")
    i32_d = nc.dram_tensor("idn32", [P, P], f32, kind="ExternalInput")
    out_d = nc.dram_tensor("outT", [D, NT], f32, kind="ExternalOutput")

    with tile.TileContext(nc) as tc:
        with tc.tile_pool(name="wpool", bufs=1) as wp, \
             tc.tile_pool(name="flux", bufs=1) as flux, \
             tc.tile_pool(name="xw", bufs=2) as xw, \
             tc.tile_pool(name="qk", bufs=1) as qk, \
             tc.tile_pool(name="big", bufs=1) as big, \
             tc.tile_pool(name="e2p", bufs=1) as e2p, \
             tc.tile_pool(name="sc", bufs=4) as sc, \
             tc.tile_pool(name="ps", bufs=2, space="PSUM") as ps:

            # wq/wk live in the slot A2 reuses after qkv (tag rotation WAR)
            wqwk = flux.tile([P, 2 * KO, 1024], bf, tag="fx")
            qb = wp.tile([P, 8], f32)
            pwb = wp.tile([P, 6], f32)
            wblk = wp.tile([P, 64], bf)
            idn = wp.tile([P, P], bf)
            idn32 = wp.tile([P, P], f32)
            # spread the big input loads over three DMA queues so the first
            # qkv matmul isn't serialized behind ~11us of transfers
            x_sb = xw.tile([P, 9, NT], bf, tag="xw")
            xf_sb = xw.tile([P, 9, NT], bf, tag="xw")
            wv = e2p.tile([P, KO, 784], bf, tag="E2")
            wqr = wq_d.rearrange("(ko p) m -> p ko m", p=P)
            wkr = wk_d.rearrange("(ko p) m -> p ko m", p=P)
            xr = x_d.rearrange("(ko p) m -> p ko m", p=P)
            xfr = xf_d.rearrange("(ko p) m -> p ko m", p=P)
            nc.sync.dma_start(x_sb[:, 0:2, :], xr[:, 0:2, :])
            nc.sync.dma_start(qb[:], qb_d[:])
            nc.sync.dma_start(wqwk[:, 0:2, :], wqr[:, 0:2, :])
            nc.sync.dma_start(x_sb[:, 2:KO, :], xr[:, 2:KO, :])
            nc.sync.dma_start(wqwk[:, 2:KO, :], wqr[:, 2:KO, :])
            nc.sync.dma_start(xf_sb[:, 0:2, :], xfr[:, 0:2, :])
            nc.sync.dma_start(xf_sb[:, 2:KO, :], xfr[:, 2:KO, :])
            nc.sync.dma_start(wv[:], wv_d.rearrange("(ko p) m -> p ko m", p=P))
            nc.sync.dma_start(pwb[:], pwb_d[:])
            # wk rides the gpsimd software-DGE queue: slower per byte but fully
            # parallel with the SP queue, so the critical x/wq prefix shrinks
            nc.gpsimd.dma_start(wqwk[:, KO:KO + 2, :], wkr[:, 0:2, :])
            nc.gpsimd.dma_start(wqwk[:, KO + 2:, :], wkr[:, 2:KO, :])
            nc.gpsimd.dma_start(wblk[:], wb_d[:])
            nc.gpsimd.dma_start(idn[:], id_d[:])
            nc.gpsimd.dma_start(idn32[:], i32_d[:])

            # ---- qkv ----
            # per-t tiles so block-0 scores can start as soon as a head pair
            # is ready (overlaps the exp phase with qkv matmuls)
            qt = [qk.tile([P, NT], bf, tag=f"qt{t}", name=f"qt{t}") for t in range(8)]
            kt = [qk.tile([P, NT], bf, tag=f"kt{t}", name=f"kt{t}") for t in range(8)]
            qft = [qk.tile([P, NT], bf, tag=f"qft{t}", name=f"qft{t}") for t in range(8)]
            kft = [qk.tile([P, NT], bf, tag=f"kft{t}", name=f"kft{t}") for t in range(8)]
            # k-bias dropped: (q+bq)@k differs from ref scores by a per-row
            # constant that cancels in softmax.  q-bias added on PSUM->SBUF copy.
            # x-phase jobs first, xf-phase second: xf's DMA lands ~13us in,
            # so all its consumers run after the x work has warmed the PE
            qkv_x = () if 'qkv' in ABLATE else (
                (qt, 0, x_sb, True), (kt, KO, x_sb, False))
            qkv_f = () if 'qkv' in ABLATE else (
                (qft, 0, xf_sb, True), (kft, KO, xf_sb, False))

            A = big.tile([P, MPAD, 32], bf)
            nc.vector.memset(A[:, NT:MPAD, :], 0.0)
            # weighted values, [n, nblk, (o hd pad16)]; pad cols zeroed once
            wtp = qk.tile([P, 5, 1024], bf)
            wtpv = wtp.rearrange("p b (o j) -> p b o j", j=64)
            nc.vector.memset(wtpv[:, :, :, 48:], 0.0)
            wtT = qk.tile([P, 9, NT], bf)

            def score_one(c, n0, nlen, zt, Ab):
                qs, ks = (qt, kt) if c < 16 else (qft, kft)
                h = c % 16
                t, off = h // 2, 64 * (h % 2)
                sp = ps.tile([P, 577], f32, tag="sp", name="sp")
                for o0, w in ((0, 512), (512, 65)):
                    nc.tensor.matmul(sp[:nlen, o0:o0 + w], qs[t][off:off + 64, ds(n0, nlen)],
                                     ks[t][off:off + 64, ds(o0, w)], start=True, stop=True)
                nc.scalar.activation(Ab[:nlen, :NT, c], sp[:nlen, :NT], EXP,
                                     scale=SCALE, accum_out=zt[:nlen, c:c + 1])

            # block-0 scores run inside the qkv loop: the exp phase of block 0
            # hides under the qkv matmuls
            zt0 = sc.tile([P, 32], f32, tag="zt")
            nc.vector.memset(zt0[:], 1.0)
            do_s0 = 'score' not in ABLATE and 'qkv' not in ABLATE
            def qkv_job(dst, kb, src, has_b, t):
                pa = ps.tile([P, 512], f32, tag="rp", name="pa")
                pb = ps.tile([P, 272], f32, tag="mx", name="pb")
                for ko in range(KO):
                    nc.tensor.matmul(pa[:, :], wqwk[:, kb + ko, ts(t, P)],
                                     src[:, ko, ds(0, 512)],
                                     start=(ko == 0), stop=(ko == KO - 1))
                    nc.tensor.matmul(pb[:, :65], wqwk[:, kb + ko, ts(t, P)],
                                     src[:, ko, ds(512, 65)],
                                     start=(ko == 0), stop=(ko == KO - 1))
                if has_b:
                    nc.vector.tensor_scalar_add(dst[t][:, :512], pa[:, :], qb[:, t:t + 1])
                    nc.vector.tensor_scalar_add(dst[t][:, 512:], pb[:, :65], qb[:, t:t + 1])
                else:
                    nc.vector.tensor_copy(dst[t][:, :512], pa[:, :])
                    nc.vector.tensor_copy(dst[t][:, 512:], pb[:, :65])

            for t in range(8):
                for dst, kb, src, has_b in qkv_x:
                    qkv_job(dst, kb, src, has_b, t)
                if do_s0:
                    for c in (2 * t, 2 * t + 1):
                        score_one(c, 0, 128, zt0, A)
            for t in range(8):
                for dst, kb, src, has_b in qkv_f:
                    qkv_job(dst, kb, src, has_b, t)
                if do_s0:
                    for c in (16 + 2 * t, 17 + 2 * t):
                        score_one(c, 0, 128, zt0, A)
            # second attention buffer in the wq/wk slot: blocks alternate A/A2
            # so block i+1's exps overlap block i's mix stage
            A2 = flux.tile([P, MPAD, 32], bf, tag="fx")
            nc.vector.memset(A2[:, NT:MPAD, :], 0.0)

            # v [tokens, 784]; v-bias folded into proj ones-row host-side
            v_sb = qk.tile([P, 5, 784], bf)
            for mt, (m0, mlen) in enumerate(NBLK):
                pa = ps.tile([P, 512], f32, tag="rp")
                pb = ps.tile([P, 272], f32, tag="mx")
                for ko in range(KO):
                    nc.tensor.matmul(pa[:mlen, :], x_sb[:, ko, ds(m0, mlen)],
                                     wv[:, ko, ds(0, 512)], start=(ko == 0), stop=(ko == KO - 1))
                    nc.tensor.matmul(pb[:mlen, :], x_sb[:, ko, ds(m0, mlen)],
                                     wv[:, ko, ds(512, 272)], start=(ko == 0), stop=(ko == KO - 1))
                nc.vector.tensor_copy(v_sb[:mlen, mt, :512], pa[:mlen, :])
                nc.vector.tensor_copy(v_sb[:mlen, mt, 512:], pb[:mlen, :])
            # ones column per head (z2 accumulator row of pv)
            v_view = v_sb.rearrange("p mt (h j) -> p mt h j", j=49)
            nc.vector.memset(v_view[:, :, :, 48], 1.0)

            pw = xw.tile([P, 8, D], bf, tag="xw")  # reuses x slot after last read
            nc.sync.dma_start(pw[:], pw_d.rearrange("(ko p) m -> p ko m", p=P))

            zt_pre = [None] * 5   # zt of block i+1, allocated early (see below)
            for bi, (n0, nlen) in enumerate(NBLK):
                Ab = A if bi % 2 == 0 else A2
                if bi == 0:
                    zt = zt0
                else:
                    zt = zt_pre[bi]
                    for c in ([] if 'score' in ABLATE else range(2, 32)):
                        score_one(c, n0, nlen, zt, Ab)
                if bi < 4 and 'score' not in ABLATE:
                    # emit the first two channels of the NEXT block here: they
                    # fill the Act bubble while this block's zirep chain runs
                    ztn = sc.tile([P, 32], f32, tag="zt", name="ztn")
                    nc.vector.memset(ztn[:], 1.0)
                    zt_pre[bi + 1] = ztn
                    An = A2 if bi % 2 == 0 else A
                    n0n, nlenn = NBLK[bi + 1]
                    for c in (0, 1):
                        score_one(c, n0n, nlenn, ztn, An)
                zi = sc.tile([P, 32], f32, tag="zi")
                nc.vector.reciprocal(zi[:], zt[:])
                # zirep[(mj c), n] = 1/z1[c, n] in bf16 (keeps rs-mul in 2x mode);
                # folded into the rp->rs copy
                zit = ps.tile([32, P], f32, tag="mx")
                nc.tensor.transpose(zit[:, :], zi[:], idn32[:])
                zirep = sc.tile([P, P], bf, tag="zir")
                for mj in range(4):
                    nc.vector.tensor_copy(zirep[ds(32 * mj, 32), :], zit[:, :])

                E2 = e2p.tile([P, 16, MPAD], bf, tag="E2")
                E2v = E2.rearrange("p o (g mj) -> p g mj o", mj=4)
                for gb in ([] if 'mix' in ABLATE else range(0, NG, 8)):
                    ng = min(8, NG - gb)
                    rp = ps.tile([P, 8, P], bf, tag="rp")
                    for gi in range(ng):
                        slab = Ab[:, ds(4 * (gb + gi), 4), :].rearrange("p m c -> p (m c)")
                        nc.tensor.transpose(rp[:, gi, :], slab, idn[:])
                    rs = sc.tile([P, 8, P], bf, tag="rsb", bufs=4)
                    zb = bass.AP(tensor=zirep.tensor, offset=zirep.offset,
                                 ap=[list(zirep.ap)[0], [0, ng], list(zirep.ap)[1]])
                    nc.vector.tensor_mul(rs[:, :ng, :], rp[:, :ng, :], zb)
                    mp = ps.tile([P, 8, 64], f32, tag="mx")
                    for gi in range(ng):
                        nc.tensor.matmul(mp[:, gi, :], rs[:, gi, :], wblk[:], start=True, stop=True)
                    mpv = mp.rearrange("p g (mj o) -> p g mj o", o=16)
                    nc.scalar.activation(E2v[:, ds(gb, ng), :, :], mpv[:, :ng, :, :], EXP)

                if bi == 4 and 'pv' not in ABLATE:
                    # proj for n-cols 0..512 (blocks 0-3 done): PE work overlaps
                    # the DVE/Act-heavy PV of block 4
                    for dt in range(6):
                        fpa = ps.tile([P, 512], f32, tag="sp")
                        for ko in range(8):
                            nc.tensor.matmul(fpa[:, :], pw[:, ko, ts(dt, P)],
                                             wtT[:, ko, ds(0, 512)], start=(ko == 0), stop=(ko == 7))
                        oba = sc.tile([P, 512], f32, tag="osb")
                        nc.vector.tensor_scalar_add(oba[:], fpa[:], pwb[:, dt:dt + 1])
                        nc.sync.dma_start(out_d[ts(dt, P), :512], oba[:])

                for o in ([] if 'pv' in ABLATE else range(16)):
                    tpb = ps.tile([P, 5, P], bf, tag="rp" if bi == 4 else "mx")
                    for mt in range(5):
                        w = P if mt < 4 else MPAD - 512
                        nc.tensor.transpose(tpb[:w, mt, :], E2[:, o, ds(128 * mt, w)], idn[:])
                    e2t = sc.tile([P, 5, P], bf, tag="e2t", bufs=4)
                    nc.vector.tensor_copy(e2t[:], tpb[:])
                    # pv[n, (hd, z2)]: n on partitions so z2 normalize is per-partition
                    pv = ps.tile([P, 49], f32, tag="mx")
                    for mt, (m0, mlen) in enumerate(NBLK):
                        nc.tensor.matmul(pv[:, :], e2t[:mlen, mt, :],
                                         v_sb[:mlen, mt, ds(49 * o, 49)],
                                         start=(mt == 0), stop=(mt == 4))
                    zr = sc.tile([P, 1], f32, tag="zri")
                    nc.vector.reciprocal(zr[:], pv[:, 48:49])
                    nc.vector.tensor_scalar_mul(wtp[:, bi, ds(64 * o, 48)], pv[:, :48], zr[:, 0:1])

                # transpose this block's weighted values to [(o hd), n] for proj
                wn = P if bi < 4 else NT - 512
                for t in range(8):
                    tp = ps.tile([P, P], bf, tag="mx")
                    nc.tensor.transpose(tp[:, :], wtp[:, bi, ds(128 * t, P)], idn[:])
                    nc.vector.tensor_copy(wtT[:, t, ds(128 * bi, wn)], tp[:, :wn])

            # ---- proj tail: last 65 n-cols (block 4), one batched store ----
            obball = sc.tile([P, 6, 65], f32, tag="osb")
            for dt in range(6):
                fpb = ps.tile([P, 65], f32, tag="mx")
                for ko in range(8):
                    nc.tensor.matmul(fpb[:, :], pw[:, ko, ts(dt, P)],
                                     wtT[:, ko, ds(512, 65)], start=(ko == 0), stop=(ko == 7))
                nc.vector.tensor_scalar_add(obball[:, dt, :], fpb[:], pwb[:, dt:dt + 1])
            nc.sync.dma_start(out_d.rearrange("(dt p) m -> p dt m", p=P)[:, :, 512:], obball[:])
    nc.finalize()
    return nc


def _prep_weights(qkv_w, qkv_b, conv_w, proj_w, proj_b):
    f = np.float32
    qkv_w, qkv_b = qkv_w.astype(f), qkv_b.astype(f)
    proj_w, proj_b = proj_w.astype(f), proj_b.astype(f)
    wq = np.zeros((D, 1024), f)
    wk = np.zeros((D, 1024), f)
    wv = np.zeros((D, 784), f)
    qbf = np.zeros(1024, f)
    for h in range(H):
        wq[:, 64 * h:64 * h + 48] = qkv_w[48 * h:48 * h + 48, :].T
        wk[:, 64 * h:64 * h + 48] = qkv_w[768 + 48 * h:768 + 48 * h + 48, :].T
        wv[:, 49 * h:49 * h + 48] = qkv_w[1536 + 48 * h:1536 + 48 * h + 48, :].T
        qbf[64 * h:64 * h + 48] = qkv_b[48 * h:48 * h + 48]
    qb = np.ascontiguousarray(qbf.reshape(8, P).T)
    pw = np.zeros((1024, D), f)
    for h in range(H):
        pw[64 * h:64 * h + 48, :] = proj_w[:, 48 * h:48 * h + 48].T
    # proj bias + folded v-bias (sum_m attn2/z2 == 1), applied on the
    # output copies as a per-partition scalar instead of a contraction row
    pbias = proj_b + proj_w @ qkv_b[1536:2304]
    pwb = np.ascontiguousarray(pbias.reshape(6, P).T)
    wblk = np.zeros((P, 64), f)
    for mj in range(4):
        wblk[32 * mj:32 * mj + 32, 16 * mj:16 * mj + 16] = conv_w.astype(f).T
    idn = np.eye(P, dtype=f)
    return {"wqT": wq.astype(BF), "wkT": wk.astype(BF), "wvT": wv.astype(BF),
            "qb": qb, "pwb": pwb, "pwT": pw.astype(BF), "wblk": wblk.astype(BF),
            "idn": idn.astype(BF), "idn32": idn}


def kernel(x, x_freq, qkv_w, qkv_b, conv_w, conv_b, proj_w, proj_b, _profile=False):
    # conv_b is constant along the softmax axis -> cancels in softmax; unused.
    if "nc" not in _cache:
        _cache["nc"] = _build(bacc.Bacc())
    nc = _cache["nc"]
    wmap = _prep_weights(np.asarray(qkv_w), np.asarray(qkv_b), np.asarray(conv_w),
                         np.asarray(proj_w), np.asarray(proj_b))
    B = x.shape[0]
    in_maps = []
    for b in range(B):
        xT = np.ascontiguousarray(np.asarray(x[b], np.float32).T).astype(BF)
        xfT = np.ascontiguousarray(np.asarray(x_freq[b], np.float32).T).astype(BF)
        in_maps.append({"xT": xT, "xfT": xfT, **wmap})
    res = run_bass_kernel_spmd(nc, in_maps, core_ids=list(range(B)), trace=_profile)
    out = np.stack([res.results[b]["outT"].T for b in range(B)], axis=0)
    if _profile:
        return out.astype(np.float32), res
    return out.astype(np.float32)
